# revision 5
# baseline (speedup 1.0000x reference)
"""BiLSTM (T=128, B=256, IN=H=512, L=3) Trainium2 Bass kernel, 8-core SPMD.

Strategy:
  - Batch sharded 8 ways (32 seqs/core); each core runs both directions of
    all 3 layers. No collectives. Forward batch occupies partitions 0-31,
    backward partitions 32-63, so one instruction covers both directions.
  - Per layer, input projections (xproj, phase A) are computed by batched
    bf16 matmuls (M-tiles of 128 rows = 4 timesteps x 32 batch) into DRAM,
    interleaved with the recurrence (phase B) so the PE stays busy during
    the serial cell chain. xproj is injected into each step's PSUM
    accumulation with an f32r identity matmul; Whh^T streams as the moving
    operand against the stationary transposed hidden state h^T (f32r).
  - Gate columns are host-permuted to [g,i,f,o] so the per-512-column PSUM
    chunks finish in dependency order of the cell math (tanh(g) first,
    sigma(o) last), letting ACT/DVE pipeline behind the PE.
  - Ragged sequences: lengths sorted descending. Forward rows past their
    length compute harmless garbage (h bounded by +-1); backward rows stay
    exactly zero until activation via a per-partition mask column fused into
    the sigma(i)*tanh(g) product. Padded outputs are zeroed on the host.
"""

import os

import ml_dtypes
import numpy as np

import concourse.bacc as bacc
import concourse.bass as bass
import concourse.mybir as mybir
import concourse.tile as tile
from concourse.bass_utils import run_bass_kernel_spmd

F32 = mybir.dt.float32
F32R = mybir.dt.float32r
BF16 = mybir.dt.bfloat16
AF = mybir.ActivationFunctionType
ALU = mybir.AluOpType

T_FULL = 128
B_FULL = 256
IN0 = 512
H = 512
L = 3
NCORES = 8
BS = B_FULL // NCORES  # 32
G4 = 4 * H  # 2048
NCH = 4  # gate chunks of 512 (order: g, i, f, o)
HC = H // 128  # h^T partition chunks

FUSE = os.environ.get("FUSE", "1") == "1"
PF = 2  # phase-A prefetch distance in step-groups

# dev override: smaller T for fast compile during bring-up (multiple of 4)
T = int(os.environ.get("KERNEL_T", str(T_FULL)))

last_results = None


def _gate_perm():
    """Rows from pytorch order (i,f,g,o) to chunk order (g,i,f,o)."""
    idx = np.arange(G4)
    return np.concatenate([idx[2 * H : 3 * H], idx[0:H], idx[H : 2 * H], idx[3 * H : 4 * H]])


# cell chunk indices after permutation
CG, CI, CF, CO = 0, 1, 2, 3


def build_program(n_steps: int):
    nc = bacc.Bacc()
    Tn = n_steps
    MT = Tn * BS // 128  # phase-A M-tiles per dir (= step groups)

    xT = nc.dram_tensor("xT", [IN0, Tn * BS], BF16, kind="ExternalInput")
    maskd = nc.dram_tensor("maskd", [64, Tn], F32, kind="ExternalInput")
    wih = {}
    whh = {}
    for l in range(L):
        ind = IN0 if l == 0 else 2 * H
        for d in ("f", "b"):
            wih[(l, d)] = nc.dram_tensor(f"wih{l}{d}", [ind, G4], BF16, kind="ExternalInput")
            whh[(l, d)] = nc.dram_tensor(f"whh{l}{d}", [H, G4], F32R, kind="ExternalInput")
    out = nc.dram_tensor("out", [Tn * BS, 2 * H], F32, kind="ExternalOutput")

    ident_d = nc.inline_tensor(np.eye(64, dtype=np.float32), name="ident")

    with tile.TileContext(nc) as tc:
        with (
            tc.tile_pool(name="const", bufs=1) as constp,
            tc.tile_pool(name="dram", bufs=2, space="DRAM") as dramp,
        ):
            i64 = constp.tile([64, 64], F32, name="i64")
            nc.sync.dma_start(i64[:], ident_d[:])
            i64r = constp.tile([64, 64], F32R, name="i64r")
            nc.vector.tensor_copy(i64r[:], i64[:])
            msk = constp.tile([64, Tn], F32, name="msk")
            nc.sync.dma_start(msk[:], maskd[:])

            xt_prev_f = None
            xt_prev_b = None
            for l in range(L):
                ind = IN0 if l == 0 else 2 * H
                KC = ind // 128

                xp_f = dramp.tile([Tn * BS, G4], F32R, name="xp_f")
                xp_b = dramp.tile([Tn * BS, G4], F32R, name="xp_b")
                if l < L - 1:
                    xt_nf = dramp.tile([H, Tn * BS], BF16, name="xt_nf")
                    xt_nb = dramp.tile([H, Tn * BS], BF16, name="xt_nb")
                else:
                    xt_nf = xt_nb = None

                def lhs_src(c):
                    if l == 0:
                        return xT[c * 128 : (c + 1) * 128, :]
                    if c < HC:
                        return xt_prev_f[c * 128 : (c + 1) * 128, :]
                    cc = c - HC
                    return xt_prev_b[cc * 128 : (cc + 1) * 128, :]

                with (
                    tc.tile_pool(name="wihp", bufs=2) as wihp,
                    tc.tile_pool(name="whhp", bufs=2) as whhp,
                    tc.tile_pool(name="lhsp", bufs=2 * KC + 2) as lhsp,
                    tc.tile_pool(name="psA", bufs=2, space="PSUM") as psA,
                    tc.tile_pool(name="outA", bufs=4) as outA,
                    tc.tile_pool(name="xpsp", bufs=2) as xpsp,
                    tc.tile_pool(name="gps", bufs=4, space="PSUM") as gps,
                    tc.tile_pool(name="tps", bufs=2, space="PSUM") as tps,
                    tc.tile_pool(name="cellp", bufs=2) as cellp,
                    tc.tile_pool(name="statep", bufs=1) as statep,
                ):
                    wsbA = {}
                    wsbB = {}
                    for d in ("f", "b"):
                        w = wihp.tile([128, KC * G4], BF16, name="wihsb")
                        for c in range(KC):
                            nc.sync.dma_start(
                                w[:, c * G4 : (c + 1) * G4],
                                wih[(l, d)][c * 128 : (c + 1) * 128, :],
                            )
                        wsbA[d] = w
                        w2 = whhp.tile([128, HC * G4], F32R, name="whhsb")
                        for c in range(HC):
                            nc.sync.dma_start(
                                w2[:, c * G4 : (c + 1) * G4],
                                whh[(l, d)][c * 128 : (c + 1) * 128, :],
                            )
                        wsbB[d] = w2

                    hTf = statep.tile([128, HC * 64], F32R, name="hTf")
                    hTb = statep.tile([128, HC * 64], F32R, name="hTb")
                    hT16 = statep.tile([128, HC * 64], BF16, name="hT16")
                    cst = statep.tile([64, H], F32, name="cst")
                    hb = statep.tile([64, H], F32, name="hb")
                    nc.vector.memset(hTf[:].bitcast(F32), 0.0)
                    nc.vector.memset(hTb[:].bitcast(F32), 0.0)
                    nc.vector.memset(hT16[:], 0.0)
                    nc.vector.memset(cst[:], 0.0)
                    nc.vector.memset(hb[:], 0.0)

                    def emit_A(d, m, xp):
                        lts = []
                        for c in range(KC):
                            lt = lhsp.tile([128, 128], BF16, name="lhst")
                            nc.sync.dma_start(lt[:], lhs_src(c)[:, m * 128 : (m + 1) * 128])
                            lts.append(lt)
                        for n in range(NCH):
                            ps = psA.tile([128, 512], F32, name="psa", space="PSUM")
                            for c in range(KC):
                                nc.tensor.matmul(
                                    ps[:],
                                    lts[c][:],
                                    wsbA[d][:, c * G4 + n * 512 : c * G4 + (n + 1) * 512],
                                    start=(c == 0),
                                    stop=(c == KC - 1),
                                )
                            so = outA.tile([128, 512], F32R, name="soA")
                            nc.vector.tensor_copy(so[:], ps[:])
                            nc.sync.dma_start(
                                xp[m * 128 : (m + 1) * 128, n * 512 : (n + 1) * 512], so[:]
                            )

                    def emit_step(s):
                        tf = s
                        tb = Tn - 1 - s
                        xps = xpsp.tile([64, G4], F32R, name="xps")
                        nc.sync.dma_start(xps[0:32, :], xp_f[tf * BS : (tf + 1) * BS, :])
                        nc.sync.dma_start(xps[32:64, :], xp_b[tb * BS : (tb + 1) * BS, :])

                        acts = [None] * NCH
                        p = q = tch = None
                        for n in range(NCH):
                            g = gps.tile([64, 512], F32, name="Gc", space="PSUM")
                            ns = slice(n * 512, (n + 1) * 512)
                            nc.tensor.matmul(
                                g[:], i64r[:], xps[:, ns], start=True, stop=False
                            )
                            for c in range(HC):
                                rh = slice(c * G4 + n * 512, c * G4 + (n + 1) * 512)
                                cs = slice(c * 64, (c + 1) * 64)
                                nc.tensor.matmul(
                                    g[:], hTf[:, cs], wsbB["f"][:, rh],
                                    start=False, stop=False,
                                )
                                nc.tensor.matmul(
                                    g[:], hTb[:, cs], wsbB["b"][:, rh],
                                    start=False, stop=(c == HC - 1),
                                )
                            a = cellp.tile([64, 512], F32, name=f"act{n}")
                            nc.scalar.activation(
                                a[:], g[:], AF.Tanh if n == CG else AF.Sigmoid
                            )
                            acts[n] = a
                            if n == CI:
                                p = cellp.tile([64, 512], F32, name="p", bufs=1)
                                nc.vector.scalar_tensor_tensor(
                                    p[:], acts[CI][:], msk[:, s : s + 1], acts[CG][:],
                                    ALU.mult, ALU.mult,
                                )
                            elif n == CF:
                                q = cellp.tile([64, 512], F32, name="q", bufs=1)
                                nc.vector.tensor_mul(q[:], acts[CF][:], cst[:])
                                nc.vector.tensor_add(cst[:], p[:], q[:])
                                tch = cellp.tile([64, 512], F32, name="tch", bufs=1)
                                nc.scalar.activation(tch[:], cst[:], AF.Tanh)
                        nc.vector.tensor_mul(hb[:], acts[CO][:], tch[:])

                        TP = tps.tile([128, HC * 64], F32, name="TP", space="PSUM")
                        for c in range(HC):
                            nc.tensor.transpose(
                                TP[:, c * 64 : (c + 1) * 64],
                                hb[:, c * 128 : (c + 1) * 128],
                                i64[:],
                            )
                        tp4 = TP[:].rearrange("p (c x) -> p c x", c=HC)
                        hf4 = hTf[:].rearrange("p (c x) -> p c x", c=HC)
                        hb4 = hTb[:].rearrange("p (c x) -> p c x", c=HC)
                        nc.vector.tensor_copy(hf4[:, :, 0:32], tp4[:, :, 0:32])
                        nc.vector.tensor_copy(hb4[:, :, 32:64], tp4[:, :, 32:64])

                        if l < L - 1:
                            nc.vector.tensor_copy(hT16[:], TP[:])
                            for c in range(HC):
                                nc.sync.dma_start(
                                    xt_nf[c * 128 : (c + 1) * 128, tf * BS : (tf + 1) * BS],
                                    hT16[:, c * 64 : c * 64 + 32],
                                )
                                nc.sync.dma_start(
                                    xt_nb[c * 128 : (c + 1) * 128, tb * BS : (tb + 1) * BS],
                                    hT16[:, c * 64 + 32 : c * 64 + 64],
                                )
                        else:
                            nc.sync.dma_start(out[tf * BS : (tf + 1) * BS, 0:H], hb[0:32, :])
                            nc.sync.dma_start(
                                out[tb * BS : (tb + 1) * BS, H : 2 * H], hb[32:64, :]
                            )

                    if FUSE:
                        for k in range(min(PF, MT)):
                            emit_A("f", k, xp_f)
                            emit_A("b", MT - 1 - k, xp_b)
                        for k in range(MT):
                            if k + PF < MT:
                                emit_A("f", k + PF, xp_f)
                                emit_A("b", MT - 1 - k - PF, xp_b)
                            for s in range(4 * k, 4 * k + 4):
                                emit_step(s)
                    else:
                        for m in range(MT):
                            emit_A("f", m, xp_f)
                            emit_A("b", m, xp_b)
                        for s in range(Tn):
                            emit_step(s)

                xt_prev_f, xt_prev_b = xt_nf, xt_nb

    nc.compile()
    return nc


def _prep_inputs(x, lengths, params, n_steps):
    Tn = n_steps
    x = np.asarray(x, dtype=np.float32)[:Tn]
    lengths = np.minimum(np.asarray(lengths).astype(np.int64), Tn)
    perm = _gate_perm()

    weights = {}
    for l, layer in enumerate(params):
        for d in ("f", "b"):
            p = {k: np.asarray(v, dtype=np.float32) for k, v in layer[d].items()}
            if np.abs(p["b"]).max() != 0:
                raise NotImplementedError("nonzero LSTM bias not supported")
            weights[f"wih{l}{d}"] = np.ascontiguousarray(p["Wih"][perm].T).astype(
                ml_dtypes.bfloat16
            )
            weights[f"whh{l}{d}"] = np.ascontiguousarray(p["Whh"][perm].T)

    in_maps = []
    for k in range(NCORES):
        sl = slice(k * BS, (k + 1) * BS)
        xs = x[:, sl, :]
        xTk = np.ascontiguousarray(xs.transpose(2, 0, 1).reshape(IN0, Tn * BS)).astype(
            ml_dtypes.bfloat16
        )
        lens = lengths[sl]
        m = np.ones((64, Tn), dtype=np.float32)
        steps = np.arange(Tn)[None, :]
        m[32:64] = (steps >= (Tn - lens[:, None])).astype(np.float32)
        im = {"xT": xTk, "maskd": m}
        im.update(weights)
        in_maps.append(im)
    return in_maps, lengths


def kernel(x, lengths, params):
    global last_results
    n_steps = T
    in_maps, lens = _prep_inputs(x, lengths, params, n_steps)
    nc = build_program(n_steps)
    res = run_bass_kernel_spmd(nc, in_maps, core_ids=list(range(NCORES)), trace=False)
    last_results = res
    outs = [r["out"].reshape(n_steps, BS, 2 * H) for r in res.results]
    full = np.concatenate(outs, axis=1)
    mask = (np.arange(n_steps)[:, None] < lens[None, :]).astype(np.float32)
    full = full * mask[:, :, None]
    return full


# revision 9
# speedup vs baseline: 1.0116x; 1.0116x over previous
"""BiLSTM (T=128, B=256, IN=H=512, L=3) Trainium2 Bass kernel, 8-core SPMD.

Strategy (v3 — direction-split):
  - Cores are paired: pair g = cores (2g, 2g+1) share batch group g (64
    seqs); the even core runs the forward direction, the odd core the
    backward direction. Direction differences are pure DATA under one SPMD
    program: backward cores receive x time-reversed, their own weights under
    the same input names, an activation mask, and the host un-reverses
    their outputs.
  - Recurrent matmuls keep the transposed hidden state h^T (f32r) as the
    PE-stationary operand (M=64) while Whh^T streams; per-step xproj is
    injected into the PSUM accumulation by an f32r identity matmul.
  - Input projections (phase A) run as batched f32r matmuls (M-tiles of
    128 rows = 2 steps x 64 batch) interleaved with the recurrence so the
    PE stays busy through the serial cell chain.
  - At each layer boundary the transposed layer outputs are exchanged
    within each pair by an AllGather; the partner half is read with a
    per-core dynamic row offset (partition_id parity) and a reversed
    time window.
  - Gate columns are host-permuted to [g,i,f,o] so PSUM chunks finish in
    cell-math dependency order, pipelining ACT/DVE behind the PE.
  - Ragged sequences: lengths sorted descending. Forward rows past their
    length compute harmless garbage (h bounded by +-1, padded outputs
    zeroed on host); backward rows stay exactly zero until activation via
    a per-partition mask column fused into the sigma(i)*tanh(g) product.
"""

import os

import numpy as np

import concourse.bacc as bacc
import concourse.bass as bass
import concourse.mybir as mybir
import concourse.tile as tile
from concourse.bass_utils import run_bass_kernel_spmd

F32 = mybir.dt.float32
F32R = mybir.dt.float32r
AF = mybir.ActivationFunctionType
ALU = mybir.AluOpType

T_FULL = 128
B_FULL = 256
IN0 = 512
H = 512
L = 3
NCORES = 8
BB = 64  # batch per core (one direction of a 64-seq group)
G4 = 4 * H  # 2048
NCH = 4  # gate chunks of 512 (order: g, i, f, o)
HC = H // 128  # h^T partition chunks

PF = int(os.environ.get("PF", "3"))  # phase-A prefetch distance in A-tiles
FUSE = os.environ.get("FUSE", "1") == "1"

T = int(os.environ.get("KERNEL_T", str(T_FULL)))

last_results = None


def _gate_perm():
    """Rows from pytorch order (i,f,g,o) to chunk order (g,i,f,o)."""
    idx = np.arange(G4)
    return np.concatenate([idx[2 * H : 3 * H], idx[0:H], idx[H : 2 * H], idx[3 * H : 4 * H]])


CG, CI, CF, CO = 0, 1, 2, 3


def build_program(n_steps: int):
    nc = bacc.Bacc()
    Tn = n_steps
    MT = Tn * BB // 128  # phase-A M-tiles (= Tn/2); tile m covers steps 2m, 2m+1

    xT = nc.dram_tensor("xT", [IN0, Tn * BB], F32R, kind="ExternalInput")
    maskd = nc.dram_tensor("maskd", [BB, Tn], F32, kind="ExternalInput")
    wih = {}
    whh = {}
    for l in range(L):
        ind = IN0 if l == 0 else 2 * H
        wih[l] = nc.dram_tensor(f"wih{l}", [ind, G4], F32R, kind="ExternalInput")
        whh[l] = nc.dram_tensor(f"whh{l}", [H, G4], F32R, kind="ExternalInput")
    out = nc.dram_tensor("out", [Tn * BB, H], F32, kind="ExternalOutput")

    ident_d = nc.inline_tensor(np.eye(BB, dtype=np.float32), name="ident")

    with tile.TileContext(nc) as tc:
        with (
            tc.tile_pool(name="const", bufs=1) as constp,
            tc.tile_pool(name="dram", bufs=2, space="DRAM") as dramp,
        ):
            i64 = constp.tile([BB, BB], F32, name="i64")
            nc.sync.dma_start(i64[:], ident_d[:])
            i64r = constp.tile([BB, BB], F32R, name="i64r")
            nc.vector.tensor_copy(i64r[:], i64[:])
            msk = constp.tile([BB, Tn], F32, name="msk")
            nc.sync.dma_start(msk[:], maskd[:])

            # partner slot (0 or 1) within the gathered pair buffer
            pid = nc.partition_id()
            poff = nc.sync.compute_val((pid + 1) % 2)

            xt_prev = None  # own transposed outputs [H, Tn*BB]
            gath_prev = None  # gathered pair [2*H, Tn*BB]
            for l in range(L):
                ind = IN0 if l == 0 else 2 * H
                KC = ind // 128

                xp = dramp.tile([Tn * BB, G4], F32R, name="xp")
                if l < L - 1:
                    xt_n = dramp.tile([H, Tn * BB], F32R, name="xt_n")
                    gath_n = dramp.tile([2 * H, Tn * BB], F32R, name="gath_n")
                else:
                    xt_n = gath_n = None

                gath_v = (
                    gath_prev[:].rearrange("(s r) (tt b) -> s r tt b", s=2, b=BB)
                    if gath_prev is not None
                    else None
                )

                with (
                    tc.tile_pool(name="wihp", bufs=1) as wihp,
                    tc.tile_pool(name="whhp", bufs=1) as whhp,
                    tc.tile_pool(name="lhsp", bufs=2 * KC + 2) as lhsp,
                    tc.tile_pool(name="psA", bufs=2, space="PSUM") as psA,
                    tc.tile_pool(name="outA", bufs=4) as outA,
                    tc.tile_pool(name="xpsp", bufs=4) as xpsp,
                    tc.tile_pool(name="gps", bufs=4, space="PSUM") as gps,
                    tc.tile_pool(name="tps", bufs=2, space="PSUM") as tps,
                    tc.tile_pool(name="cellp", bufs=2) as cellp,
                    tc.tile_pool(name="statep", bufs=1) as statep,
                ):
                    wsbA = wihp.tile([128, KC * G4], F32R, name="wihsb")
                    for c in range(KC):
                        nc.sync.dma_start(
                            wsbA[:, c * G4 : (c + 1) * G4],
                            wih[l][c * 128 : (c + 1) * 128, :],
                        )
                    wsbB = whhp.tile([128, HC * G4], F32R, name="whhsb")
                    for c in range(HC):
                        nc.sync.dma_start(
                            wsbB[:, c * G4 : (c + 1) * G4],
                            whh[l][c * 128 : (c + 1) * 128, :],
                        )

                    hT = statep.tile([128, HC * BB], F32R, name="hT")
                    cst = statep.tile([BB, H], F32, name="cst")
                    hb = statep.tile([BB, H], F32, name="hb")
                    nc.vector.memset(hT[:].bitcast(F32), 0.0)
                    nc.vector.memset(cst[:], 0.0)
                    nc.vector.memset(hb[:], 0.0)

                    def emit_A(m):
                        lts = []
                        for c in range(KC):
                            lt = lhsp.tile([128, 128], F32R, name="lhst")
                            if l == 0:
                                nc.sync.dma_start(
                                    lt[:], xT[c * 128 : (c + 1) * 128, m * 128 : (m + 1) * 128]
                                )
                            elif c < HC:
                                nc.sync.dma_start(
                                    lt[:],
                                    xt_prev[c * 128 : (c + 1) * 128, m * 128 : (m + 1) * 128],
                                )
                            else:
                                cc = c - HC
                                u0 = Tn - 1 - 2 * m
                                lt_v = lt[:].rearrange("p (tt b) -> p tt b", tt=2)
                                if u0 >= 2:
                                    src = gath_v[
                                        bass.ds(poff, 1),
                                        cc * 128 : (cc + 1) * 128,
                                        u0 : u0 - 2 : -1,
                                        :,
                                    ]
                                else:
                                    src = gath_v[
                                        bass.ds(poff, 1),
                                        cc * 128 : (cc + 1) * 128,
                                        u0::-1,
                                        :,
                                    ]
                                nc.sync.dma_start(lt_v[:, :, :], src)
                            lts.append(lt)
                        for n in range(NCH):
                            ps = psA.tile([128, 512], F32, name="psa", space="PSUM")
                            for c in range(KC):
                                nc.tensor.matmul(
                                    ps[:],
                                    lts[c][:],
                                    wsbA[:, c * G4 + n * 512 : c * G4 + (n + 1) * 512],
                                    start=(c == 0),
                                    stop=(c == KC - 1),
                                )
                            so = outA.tile([128, 512], F32R, name="soA")
                            nc.vector.tensor_copy(so[:], ps[:])
                            nc.sync.dma_start(
                                xp[m * 128 : (m + 1) * 128, n * 512 : (n + 1) * 512], so[:]
                            )

                    def emit_step(s):
                        xps = xpsp.tile([BB, G4], F32R, name="xps")
                        nc.sync.dma_start(xps[:], xp[s * BB : (s + 1) * BB, :])

                        acts = [None] * NCH
                        p = q = tch = None
                        for n in range(NCH):
                            g = gps.tile([BB, 512], F32, name="Gc", space="PSUM")
                            ns = slice(n * 512, (n + 1) * 512)
                            for c in range(HC):
                                rh = slice(c * G4 + n * 512, c * G4 + (n + 1) * 512)
                                cs = slice(c * BB, (c + 1) * BB)
                                nc.tensor.matmul(
                                    g[:], hT[:, cs], wsbB[:, rh],
                                    start=(c == 0), stop=False,
                                )
                            nc.tensor.matmul(
                                g[:], i64r[:], xps[:, ns], start=False, stop=True
                            )
                            a = cellp.tile([BB, 512], F32, name=f"act{n}")
                            nc.scalar.activation(
                                a[:], g[:], AF.Tanh if n == CG else AF.Sigmoid
                            )
                            acts[n] = a
                            if n == CI:
                                p = cellp.tile([BB, 512], F32, name="p", bufs=1)
                                nc.vector.scalar_tensor_tensor(
                                    p[:], acts[CI][:], msk[:, s : s + 1], acts[CG][:],
                                    ALU.mult, ALU.mult,
                                )
                            elif n == CF:
                                q = cellp.tile([BB, 512], F32, name="q", bufs=1)
                                nc.vector.tensor_mul(q[:], acts[CF][:], cst[:])
                                nc.vector.tensor_add(cst[:], p[:], q[:])
                                tch = cellp.tile([BB, 512], F32, name="tch", bufs=1)
                                nc.scalar.activation(tch[:], cst[:], AF.Tanh)
                        nc.vector.tensor_mul(hb[:], acts[CO][:], tch[:])

                        TP = tps.tile([128, HC * BB], F32, name="TP", space="PSUM")
                        for c in range(HC):
                            nc.tensor.transpose(
                                TP[:, c * BB : (c + 1) * BB],
                                hb[:, c * 128 : (c + 1) * 128],
                                i64[:],
                            )
                        nc.vector.tensor_copy(hT[:], TP[:])

                        if l < L - 1:
                            for c in range(HC):
                                nc.sync.dma_start(
                                    xt_n[c * 128 : (c + 1) * 128, s * BB : (s + 1) * BB],
                                    hT[:, c * BB : (c + 1) * BB],
                                )
                        else:
                            nc.sync.dma_start(out[s * BB : (s + 1) * BB, :], hb[:])

                    if FUSE:
                        for k in range(min(PF, MT)):
                            emit_A(k)
                        for k in range(MT):
                            if k + PF < MT:
                                emit_A(k + PF)
                            for s in (2 * k, 2 * k + 1):
                                emit_step(s)
                    else:
                        for m in range(MT):
                            emit_A(m)
                        for s in range(Tn):
                            emit_step(s)

                    if l < L - 1:
                        nc.gpsimd.collective_compute(
                            "AllGather",
                            mybir.AluOpType.bypass,
                            replica_groups=[[0, 1], [2, 3], [4, 5], [6, 7]],
                            ins=[xt_n.opt()],
                            outs=[gath_n.opt()],
                        )

                xt_prev, gath_prev = xt_n, gath_n

    nc.compile()
    return nc


def _prep_inputs(x, lengths, params, n_steps):
    Tn = n_steps
    x = np.asarray(x, dtype=np.float32)[:Tn]
    lengths = np.minimum(np.asarray(lengths).astype(np.int64), Tn)
    perm = _gate_perm()

    wT = {}
    for l, layer in enumerate(params):
        for d in ("f", "b"):
            p = {k: np.asarray(v, dtype=np.float32) for k, v in layer[d].items()}
            if np.abs(p["b"]).max() != 0:
                raise NotImplementedError("nonzero LSTM bias not supported")
            wT[(l, d, "ih")] = np.ascontiguousarray(p["Wih"][perm].T)
            wT[(l, d, "hh")] = np.ascontiguousarray(p["Whh"][perm].T)

    in_maps = []
    for k in range(NCORES):
        g = k // 2
        is_b = k % 2 == 1
        d = "b" if is_b else "f"
        sl = slice(g * BB, (g + 1) * BB)
        xs = x[:, sl, :]
        if is_b:
            xs = xs[::-1]
        xTk = np.ascontiguousarray(xs.transpose(2, 0, 1).reshape(IN0, Tn * BB))
        lens = lengths[sl]
        if is_b:
            m = (np.arange(Tn)[None, :] >= (Tn - lens[:, None])).astype(np.float32)
        else:
            m = np.ones((BB, Tn), dtype=np.float32)
        im = {"xT": xTk, "maskd": np.ascontiguousarray(m)}
        for l in range(L):
            w = wT[(l, d, "ih")]
            if l > 0 and is_b:
                w = np.concatenate([w[H : 2 * H], w[0:H]], axis=0)
            im[f"wih{l}"] = np.ascontiguousarray(w)
            im[f"whh{l}"] = wT[(l, d, "hh")]
        in_maps.append(im)
    return in_maps, lengths


def assemble(core_outs, lens, n_steps):
    """core_outs: list of 8 per-core out arrays [n_steps*BB, H]."""
    full = np.zeros((n_steps, B_FULL, 2 * H), dtype=np.float32)
    for k in range(NCORES):
        g = k // 2
        o = np.asarray(core_outs[k]).reshape(n_steps, BB, H)
        if k % 2 == 1:
            full[:, g * BB : (g + 1) * BB, H : 2 * H] = o[::-1]
        else:
            full[:, g * BB : (g + 1) * BB, 0:H] = o
    mask = (np.arange(n_steps)[:, None] < lens[None, :]).astype(np.float32)
    return full * mask[:, :, None]


def kernel(x, lengths, params):
    global last_results
    n_steps = T
    in_maps, lens = _prep_inputs(x, lengths, params, n_steps)
    nc = build_program(n_steps)
    res = run_bass_kernel_spmd(nc, in_maps, core_ids=list(range(NCORES)), trace=False)
    last_results = res
    return assemble([r["out"] for r in res.results], lens, n_steps)


# revision 10
# speedup vs baseline: 2.0062x; 1.9832x over previous
"""BiLSTM (T=128, B=256, IN=H=512, L=3) Trainium2 Bass kernel, 8-core SPMD.

Strategy (v3 — direction-split):
  - Cores are paired: pair g = cores (2g, 2g+1) share batch group g (64
    seqs); the even core runs the forward direction, the odd core the
    backward direction. Direction differences are pure DATA under one SPMD
    program: backward cores receive x time-reversed, their own weights under
    the same input names, an activation mask, and the host un-reverses
    their outputs.
  - Recurrent matmuls keep the transposed hidden state h^T (f32r) as the
    PE-stationary operand (M=64) while Whh^T streams; per-step xproj is
    injected into the PSUM accumulation by an f32r identity matmul.
  - Input projections (phase A) run as batched f32r matmuls (M-tiles of
    128 rows = 2 steps x 64 batch) interleaved with the recurrence so the
    PE stays busy through the serial cell chain.
  - At each layer boundary the transposed layer outputs are exchanged
    within each pair by an AllGather; the partner half is read with a
    per-core dynamic row offset (partition_id parity) and a reversed
    time window.
  - Gate columns are host-permuted to [g,i,f,o] so PSUM chunks finish in
    cell-math dependency order, pipelining ACT/DVE behind the PE.
  - Ragged sequences: lengths sorted descending. Forward rows past their
    length compute harmless garbage (h bounded by +-1, padded outputs
    zeroed on host); backward rows stay exactly zero until activation via
    a per-partition mask column fused into the sigma(i)*tanh(g) product.
"""

import os

import numpy as np

import concourse.bacc as bacc
import concourse.bass as bass
import concourse.mybir as mybir
import concourse.tile as tile
from concourse.bass_utils import run_bass_kernel_spmd

F32 = mybir.dt.float32
F32R = mybir.dt.float32r
BF16 = mybir.dt.bfloat16
AF = mybir.ActivationFunctionType
ALU = mybir.AluOpType

T_FULL = 128
B_FULL = 256
IN0 = 512
H = 512
L = 3
NCORES = 8
BB = 64  # batch per core (one direction of a 64-seq group)
G4 = 4 * H  # 2048
NCH = 4  # gate chunks of 512 (order: g, i, f, o)
HC = H // 128  # h^T partition chunks

PF = int(os.environ.get("PF", "3"))  # phase-A prefetch distance in A-tiles
FUSE = os.environ.get("FUSE", "1") == "1"

T = int(os.environ.get("KERNEL_T", str(T_FULL)))

last_results = None


def _gate_perm():
    """Rows from pytorch order (i,f,g,o) to chunk order (g,i,f,o)."""
    idx = np.arange(G4)
    return np.concatenate([idx[2 * H : 3 * H], idx[0:H], idx[H : 2 * H], idx[3 * H : 4 * H]])


CG, CI, CF, CO = 0, 1, 2, 3


def build_program(n_steps: int):
    nc = bacc.Bacc()
    Tn = n_steps
    MT = Tn * BB // 128  # phase-A M-tiles (= Tn/2); tile m covers steps 2m, 2m+1

    xT = nc.dram_tensor("xT", [IN0, Tn * BB], F32R, kind="ExternalInput")
    maskd = nc.dram_tensor("maskd", [BB, Tn], F32, kind="ExternalInput")
    wih = {}
    wihp_ = {}
    whh = {}
    for l in range(L):
        wih[l] = nc.dram_tensor(
            f"wih{l}", [IN0 if l == 0 else H, G4], F32R if l == 0 else BF16,
            kind="ExternalInput",
        )
        if l > 0:
            wihp_[l] = nc.dram_tensor(f"wihp{l}", [H, G4], BF16, kind="ExternalInput")
        whh[l] = nc.dram_tensor(f"whh{l}", [H, G4], F32R, kind="ExternalInput")
    out = nc.dram_tensor("out", [Tn * BB, H], F32, kind="ExternalOutput")

    ident_d = nc.inline_tensor(np.eye(BB, dtype=np.float32), name="ident")

    with tile.TileContext(nc) as tc:
        with (
            tc.tile_pool(name="const", bufs=1) as constp,
            tc.tile_pool(name="dram", bufs=2, space="DRAM") as dramp,
        ):
            i64 = constp.tile([BB, BB], F32, name="i64")
            nc.sync.dma_start(i64[:], ident_d[:])
            i64r = constp.tile([BB, BB], F32R, name="i64r")
            nc.vector.tensor_copy(i64r[:], i64[:])
            msk = constp.tile([BB, Tn], F32, name="msk")
            nc.sync.dma_start(msk[:], maskd[:])

            # partner slot (0 or 1) within the gathered pair buffer
            pid = nc.partition_id()
            poff = nc.sync.compute_val((pid + 1) % 2)

            xt_prev = None  # own transposed outputs [H, Tn*BB]
            gath_prev = None  # gathered pair [2*H, Tn*BB]
            for l in range(L):
                ind = IN0 if l == 0 else 2 * H
                KC = ind // 128

                xp = dramp.tile([Tn * BB, G4], F32R, name="xp")
                if l < L - 1:
                    xt_n = dramp.tile([H, Tn * BB], BF16, name="xt_n")
                    gath_n = dramp.tile([2 * H, Tn * BB], BF16, name="gath_n")
                else:
                    xt_n = gath_n = None

                gath_v = (
                    gath_prev[:].rearrange("(s r) (tt b) -> s r tt b", s=2, b=BB)
                    if gath_prev is not None
                    else None
                )

                with (
                    tc.tile_pool(name="wihp", bufs=1) as wihp,
                    tc.tile_pool(name="whhp", bufs=1) as whhp,
                    tc.tile_pool(name="lhsp", bufs=2 * KC + 2) as lhsp,
                    tc.tile_pool(name="psA", bufs=2, space="PSUM") as psA,
                    tc.tile_pool(name="outA", bufs=4) as outA,
                    tc.tile_pool(name="xpsp", bufs=4) as xpsp,
                    tc.tile_pool(name="gps", bufs=4, space="PSUM") as gps,
                    tc.tile_pool(name="tps", bufs=2, space="PSUM") as tps,
                    tc.tile_pool(name="cellp", bufs=2) as cellp,
                    tc.tile_pool(name="statep", bufs=1) as statep,
                ):
                    KCO = HC if l > 0 else KC  # own-part k-chunks
                    wsbA = wihp.tile([128, KCO * G4], F32R if l == 0 else BF16, name="wihsb")
                    for c in range(KCO):
                        nc.sync.dma_start(
                            wsbA[:, c * G4 : (c + 1) * G4],
                            wih[l][c * 128 : (c + 1) * 128, :],
                        )
                    if l > 0:
                        wsbP = wihp.tile([128, HC * G4], BF16, name="wihsbp")
                        for c in range(HC):
                            nc.sync.dma_start(
                                wsbP[:, c * G4 : (c + 1) * G4],
                                wihp_[l][c * 128 : (c + 1) * 128, :],
                            )
                    wsbB = whhp.tile([128, HC * G4], F32R, name="whhsb")
                    for c in range(HC):
                        nc.sync.dma_start(
                            wsbB[:, c * G4 : (c + 1) * G4],
                            whh[l][c * 128 : (c + 1) * 128, :],
                        )

                    hT = statep.tile([128, HC * BB], F32R, name="hT")
                    hT16 = statep.tile([128, HC * BB], BF16, name="hT16")
                    cst = statep.tile([BB, H], F32, name="cst")
                    hb = statep.tile([BB, H], F32, name="hb")
                    nc.vector.memset(hT[:].bitcast(F32), 0.0)
                    nc.vector.memset(cst[:], 0.0)
                    nc.vector.memset(hb[:], 0.0)

                    def emit_A(m):
                        lts = []
                        for c in range(KC):
                            if l == 0:
                                lt = lhsp.tile([128, 128], F32R, name="lhst")
                                nc.sync.dma_start(
                                    lt[:], xT[c * 128 : (c + 1) * 128, m * 128 : (m + 1) * 128]
                                )
                            elif c < HC:
                                lt = lhsp.tile([128, 128], BF16, name="lhst16")
                                nc.sync.dma_start(
                                    lt[:],
                                    xt_prev[c * 128 : (c + 1) * 128, m * 128 : (m + 1) * 128],
                                )
                            else:
                                lt = lhsp.tile([128, 128], BF16, name="lhst16")
                                cc = c - HC
                                u0 = Tn - 1 - 2 * m
                                lt_v = lt[:].rearrange("p (tt b) -> p tt b", tt=2)
                                if u0 >= 2:
                                    src = gath_v[
                                        bass.ds(poff, 1),
                                        cc * 128 : (cc + 1) * 128,
                                        u0 : u0 - 2 : -1,
                                        :,
                                    ]
                                else:
                                    src = gath_v[
                                        bass.ds(poff, 1),
                                        cc * 128 : (cc + 1) * 128,
                                        u0::-1,
                                        :,
                                    ]
                                nc.sync.dma_start(lt_v[:, :, :], src)
                            lts.append(lt)
                        for n in range(NCH):
                            ps = psA.tile([128, 512], F32, name="psa", space="PSUM")
                            for c in range(KC):
                                if l == 0:
                                    w_ = wsbA[:, c * G4 + n * 512 : c * G4 + (n + 1) * 512]
                                elif c < HC:
                                    w_ = wsbA[:, c * G4 + n * 512 : c * G4 + (n + 1) * 512]
                                else:
                                    cc = c - HC
                                    w_ = wsbP[:, cc * G4 + n * 512 : cc * G4 + (n + 1) * 512]
                                nc.tensor.matmul(
                                    ps[:],
                                    lts[c][:],
                                    w_,
                                    start=(c == 0),
                                    stop=(c == KC - 1),
                                )
                            so = outA.tile([128, 512], F32R, name="soA")
                            nc.scalar.copy(so[:], ps[:])
                            nc.sync.dma_start(
                                xp[m * 128 : (m + 1) * 128, n * 512 : (n + 1) * 512], so[:]
                            )

                    def emit_step(s):
                        xps = xpsp.tile([BB, G4], F32R, name="xps")
                        nc.sync.dma_start(xps[:], xp[s * BB : (s + 1) * BB, :])

                        acts = [None] * NCH
                        p = q = tch = None
                        for n in range(NCH):
                            g = gps.tile([BB, 512], F32, name="Gc", space="PSUM")
                            ns = slice(n * 512, (n + 1) * 512)
                            for c in range(HC):
                                rh = slice(c * G4 + n * 512, c * G4 + (n + 1) * 512)
                                cs = slice(c * BB, (c + 1) * BB)
                                nc.tensor.matmul(
                                    g[:], hT[:, cs], wsbB[:, rh],
                                    start=(c == 0), stop=False,
                                )
                            nc.tensor.matmul(
                                g[:], i64r[:], xps[:, ns], start=False, stop=True
                            )
                            a = cellp.tile([BB, 512], F32, name=f"act{n}")
                            nc.scalar.activation(
                                a[:], g[:], AF.Tanh if n == CG else AF.Sigmoid
                            )
                            acts[n] = a
                            if n == CI:
                                p = cellp.tile([BB, 512], F32, name="p", bufs=1)
                                nc.vector.scalar_tensor_tensor(
                                    p[:], acts[CI][:], msk[:, s : s + 1], acts[CG][:],
                                    ALU.mult, ALU.mult,
                                )
                            elif n == CF:
                                q = cellp.tile([BB, 512], F32, name="q", bufs=1)
                                nc.vector.tensor_mul(q[:], acts[CF][:], cst[:])
                                nc.vector.tensor_add(cst[:], p[:], q[:])
                                tch = cellp.tile([BB, 512], F32, name="tch", bufs=1)
                                nc.scalar.activation(tch[:], cst[:], AF.Tanh)
                        nc.vector.tensor_mul(hb[:], acts[CO][:], tch[:])

                        TP = tps.tile([128, HC * BB], F32, name="TP", space="PSUM")
                        for c in range(HC):
                            nc.tensor.transpose(
                                TP[:, c * BB : (c + 1) * BB],
                                hb[:, c * 128 : (c + 1) * 128],
                                i64[:],
                            )
                        nc.vector.tensor_copy(hT[:], TP[:])

                        if l < L - 1:
                            nc.scalar.copy(hT16[:], TP[:])
                            for c in range(HC):
                                nc.sync.dma_start(
                                    xt_n[c * 128 : (c + 1) * 128, s * BB : (s + 1) * BB],
                                    hT16[:, c * BB : (c + 1) * BB],
                                )
                        else:
                            nc.sync.dma_start(out[s * BB : (s + 1) * BB, :], hb[:])

                    if FUSE:
                        for k in range(min(PF, MT)):
                            emit_A(k)
                        for k in range(MT):
                            if k + PF < MT:
                                emit_A(k + PF)
                            for s in (2 * k, 2 * k + 1):
                                emit_step(s)
                    else:
                        for m in range(MT):
                            emit_A(m)
                        for s in range(Tn):
                            emit_step(s)

                    if l < L - 1:
                        nc.gpsimd.collective_compute(
                            "AllGather",
                            mybir.AluOpType.bypass,
                            replica_groups=[[0, 1], [2, 3], [4, 5], [6, 7]],
                            ins=[xt_n.opt()],
                            outs=[gath_n.opt()],
                        )

                xt_prev, gath_prev = xt_n, gath_n

    nc.compile()
    return nc


def _prep_inputs(x, lengths, params, n_steps):
    Tn = n_steps
    x = np.asarray(x, dtype=np.float32)[:Tn]
    lengths = np.minimum(np.asarray(lengths).astype(np.int64), Tn)
    perm = _gate_perm()

    wT = {}
    for l, layer in enumerate(params):
        for d in ("f", "b"):
            p = {k: np.asarray(v, dtype=np.float32) for k, v in layer[d].items()}
            if np.abs(p["b"]).max() != 0:
                raise NotImplementedError("nonzero LSTM bias not supported")
            wT[(l, d, "ih")] = np.ascontiguousarray(p["Wih"][perm].T)
            wT[(l, d, "hh")] = np.ascontiguousarray(p["Whh"][perm].T)

    in_maps = []
    for k in range(NCORES):
        g = k // 2
        is_b = k % 2 == 1
        d = "b" if is_b else "f"
        sl = slice(g * BB, (g + 1) * BB)
        xs = x[:, sl, :]
        if is_b:
            xs = xs[::-1]
        xTk = np.ascontiguousarray(xs.transpose(2, 0, 1).reshape(IN0, Tn * BB))
        lens = lengths[sl]
        if is_b:
            m = (np.arange(Tn)[None, :] >= (Tn - lens[:, None])).astype(np.float32)
        else:
            m = np.ones((BB, Tn), dtype=np.float32)
        im = {"xT": xTk, "maskd": np.ascontiguousarray(m)}
        import ml_dtypes

        for l in range(L):
            w = wT[(l, d, "ih")]
            if l == 0:
                im[f"wih{l}"] = np.ascontiguousarray(w)
            else:
                own = w[H : 2 * H] if is_b else w[0:H]
                par = w[0:H] if is_b else w[H : 2 * H]
                im[f"wih{l}"] = np.ascontiguousarray(own).astype(ml_dtypes.bfloat16)
                im[f"wihp{l}"] = np.ascontiguousarray(par).astype(ml_dtypes.bfloat16)
            im[f"whh{l}"] = wT[(l, d, "hh")]
        in_maps.append(im)
    return in_maps, lengths


def assemble(core_outs, lens, n_steps):
    """core_outs: list of 8 per-core out arrays [n_steps*BB, H]."""
    full = np.zeros((n_steps, B_FULL, 2 * H), dtype=np.float32)
    for k in range(NCORES):
        g = k // 2
        o = np.asarray(core_outs[k]).reshape(n_steps, BB, H)
        if k % 2 == 1:
            full[:, g * BB : (g + 1) * BB, H : 2 * H] = o[::-1]
        else:
            full[:, g * BB : (g + 1) * BB, 0:H] = o
    mask = (np.arange(n_steps)[:, None] < lens[None, :]).astype(np.float32)
    return full * mask[:, :, None]


def kernel(x, lengths, params):
    global last_results
    n_steps = T
    in_maps, lens = _prep_inputs(x, lengths, params, n_steps)
    nc = build_program(n_steps)
    res = run_bass_kernel_spmd(nc, in_maps, core_ids=list(range(NCORES)), trace=False)
    last_results = res
    return assemble([r["out"] for r in res.results], lens, n_steps)
